# revision 34
# baseline (speedup 1.0000x reference)
"""Multi-head attention (ViT-style, RoPE) Trainium2 Bass kernel.

Problem: x[32,577,768], 12 heads, d=64, RoPE on tokens 1..576, softmax,
output projection.  Data-parallel over batch across 8 NeuronCores
(4 batches per core).  All matmuls in bf16 with fp32 PSUM accumulation.

Layout strategy per core (b_loc=4, n=577, e=768):
  xT      [768, 2308]  (tokens of 4 batches concatenated, transposed)
  Q^T,K^T [e_out, tok] computed directly (lhsT = W^T tiles, rhs = xT tiles)
  RoPE in a permuted head layout (per head: rows 0..31 = even comps,
  32..63 = odd comps) so the rotation becomes
      q_rope = CA * q + CB * (P_swap @ q)
  with CA/CB [128,577] host-precomputed coefficient tiles (token 0 rows are
  identity) and P_swap a constant 128x128 permutation done on the PE.
  energy^T[k,q] = (K^T).T @ Q^T per (b,h) in 128-row k chunks; exp on ACT
  (scale=1/sqrt(768) folded in); PV via lhsT=[V | 1] augmented with a ones
  column so row 64 of the PV accumulator is the softmax denominator.
  Normalization: reciprocal on DVE, broadcast 1->64 rows via rank-1 matmul,
  multiply on DVE.  Output projection back to [tok, e] layout with bias
  added as a rank-1 (ones x bias) accumulation matmul.
  Biases: bq/bk via ACT bias during PSUM->SBUF copy; bv folded into the
  output bias on host (out += Wp @ bv, exact because softmax weights sum
  to 1); bp via the rank-1 matmul.
"""

import numpy as np

H = 12
E = 768
D = 64
N = 577
NCORES = 8
B = 32
BL = B // NCORES          # batches per core
T = BL * N                # tokens per core
KO = E // 128             # 6 contraction chunks
SCALE = 1.0 / np.sqrt(np.float32(E))

_CACHE = {}


def _head_perm():
    perm = np.empty(E, np.int64)
    j = np.arange(64)
    local = np.where(j < 32, 2 * j, 2 * (j - 32) + 1)
    for h in range(H):
        perm[h * 64 + j] = h * 64 + local
    return perm


def _rope_coeffs(pe):
    # CA/CB [128, 577] in the permuted layout; pattern identical for the two
    # heads in a 128-partition chunk.
    ca = np.zeros((128, N), np.float32)
    cb = np.zeros((128, N), np.float32)
    ca[:, 0] = 1.0
    for hh in range(2):
        base = hh * 64
        j = np.arange(32)
        # even-output rows (local j < 32): out = pe[...,0,0]*q + pe[...,0,1]*qswap
        ca[base + j, 1:] = pe[:, :, 0, 0].T
        cb[base + j, 1:] = pe[:, :, 0, 1].T
        # odd-output rows: q holds x_odd -> coeff pe[...,1,1]; qswap holds x_even
        ca[base + 32 + j, 1:] = pe[:, :, 1, 1].T
        cb[base + 32 + j, 1:] = pe[:, :, 1, 0].T
    return ca, cb


def _build_bass():
    import concourse.bass as bass
    import concourse.mybir as mybir
    import concourse.tile as tile
    from concourse import bacc

    f32 = mybir.dt.float32
    bf16 = mybir.dt.bfloat16

    nc = bacc.Bacc("TRN2", target_bir_lowering=False, debug=False,
                   num_devices=NCORES)

    xT = nc.dram_tensor("xT", [E, T], f32, kind="ExternalInput")
    wq = nc.dram_tensor("wqT", [E, E], f32, kind="ExternalInput")
    wk = nc.dram_tensor("wkT", [E, E], f32, kind="ExternalInput")
    wv = nc.dram_tensor("wvT", [E, E], f32, kind="ExternalInput")
    wp = nc.dram_tensor("wpT", [E, E], f32, kind="ExternalInput")
    bqd = nc.dram_tensor("bqp", [128, KO], f32, kind="ExternalInput")
    bkd = nc.dram_tensor("bkp", [128, KO], f32, kind="ExternalInput")
    bped = nc.dram_tensor("bpe", [1, E], f32, kind="ExternalInput")
    cad = nc.dram_tensor("ca", [128, N], f32, kind="ExternalInput")
    cbd = nc.dram_tensor("cb", [128, N], f32, kind="ExternalInput")
    out_d = nc.dram_tensor("out", [T, E], f32, kind="ExternalOutput")

    NT = [(0, 512), (512, 65)]        # token free-dim tiles within a batch
    ET = [(0, 512), (512, 256)]       # e_out free-dim tiles
    KC = [(c * 128, 128 if c < 4 else N - 512) for c in range(5)]  # k chunks

    with tile.TileContext(nc) as tc:
        with (
            tc.tile_pool(name="const", bufs=1) as constp,
            tc.tile_pool(name="stage", bufs=2) as stagep,
            tc.tile_pool(name="persist", bufs=1) as persp,
            tc.tile_pool(name="qk", bufs=2) as qkp,
            tc.tile_pool(name="vpool", bufs=2) as vp,
            tc.tile_pool(name="otp", bufs=2) as otp,
            tc.tile_pool(name="tmp", bufs=3) as tmpp,
            tc.tile_pool(name="expp", bufs=3) as expp,
            tc.tile_pool(name="small", bufs=2) as smallp,
            tc.tile_pool(name="outsb", bufs=2) as outsbp,
            tc.tile_pool(name="ps", bufs=2, space="PSUM") as psp,
            tc.tile_pool(name="pvps", bufs=2, space="PSUM") as pvpsp,
        ):
            def psum(p, f):
                # 2-bank slot, shared by ST/projection/output tiles
                t = psp.tile([128, 768], f32, tag="ps")
                return t[:p, :f]

            def psum_pv(p, f):
                # dedicated rotation for PV accumulators
                t = pvpsp.tile([128, 768], f32, tag="pv")
                return t[:p, :f]

            # ---- load + cast constants ----
            # x and Wq chunks first (the first projection needs them), the
            # rest behind.
            xbf = persp.tile([128, KO, T], bf16)
            wtiles = {n: persp.tile([128, KO, E], bf16, tag=f"w{n}",
                                    name=f"w{n}")
                      for n in ("q", "k", "v", "p")}
            wdram = {"q": wq, "k": wk, "v": wv, "p": wp}
            for ko in range(KO):
                st = stagep.tile([128, T], f32, tag="stx")
                nc.sync.dma_start(st[:], xT[ko * 128:(ko + 1) * 128, :])
                nc.vector.tensor_copy(xbf[:, ko, :], st[:])
                sw = stagep.tile([128, E], f32, tag="stw")
                nc.sync.dma_start(sw[:], wq[ko * 128:(ko + 1) * 128, :])
                nc.vector.tensor_copy(wtiles["q"][:, ko, :], sw[:])
            for name in ("k", "v", "p"):
                wt, wd = wtiles[name], wdram[name]
                for ko in range(KO):
                    st = stagep.tile([128, E], f32, tag="stw")
                    nc.sync.dma_start(st[:], wd[ko * 128:(ko + 1) * 128, :])
                    nc.vector.tensor_copy(wt[:, ko, :], st[:])

            ca_bf = constp.tile([128, N], bf16)
            cb_bf = constp.tile([128, N], bf16)
            for dst, src in ((ca_bf, cad), (cb_bf, cbd)):
                st = stagep.tile([128, N], f32, tag="stc")
                nc.sync.dma_start(st[:], src[:, :])
                nc.vector.tensor_copy(dst[:], st[:])
            bq_sb = constp.tile([128, KO], f32, tag="bq")
            bk_sb = constp.tile([128, KO], f32, tag="bk")
            nc.sync.dma_start(bq_sb[:], bqd[:, :])
            nc.sync.dma_start(bk_sb[:], bkd[:, :])
            bpe_sb = constp.tile([1, E], bf16, tag="bpe")
            st = stagep.tile([1, E], f32, tag="stb")
            nc.sync.dma_start(st[:], bped[:, :])
            nc.vector.tensor_copy(bpe_sb[:], st[:])
            ones_sb = constp.tile([65, 128], bf16, tag="ones")
            nc.vector.memset(ones_sb[:], 1.0)

            Ident = mybir.ActivationFunctionType.Identity
            Copy = mybir.ActivationFunctionType.Copy
            Exp = mybir.ActivationFunctionType.Exp

            for b in range(BL):
                t0 = b * N

                # ---- Q^T / K^T projection + RoPE ----
                qro = qkp.tile([128, KO, N], bf16, tag="qro")
                kro = qkp.tile([128, KO, N], bf16, tag="kro")
                for wname, bias_sb, dst in (("q", bq_sb, qro), ("k", bk_sb, kro)):
                    wt = wtiles[wname]
                    for m in range(KO):
                        q_sb = tmpp.tile([128, N], bf16, tag="q_sb")
                        pq = psum_pv(128, N)
                        for ns, nw in NT:
                            for kk in range(KO):
                                nc.tensor.matmul(
                                    pq[:, ns:ns + nw],
                                    lhsT=wt[:, kk, m * 128:(m + 1) * 128],
                                    rhs=xbf[:, kk, t0 + ns:t0 + ns + nw],
                                    start=(kk == 0), stop=(kk == KO - 1),
                                )
                        nc.scalar.activation(q_sb[:], pq[:, :], Ident,
                                             bias=bias_sb[:, m:m + 1])
                        # swap even/odd 32-row blocks via SBUF->SBUF DMA
                        qsw = tmpp.tile([128, N], bf16, tag="qsw")
                        for a in (0, 64):
                            nc.gpsimd.dma_start(qsw[a:a + 32, :],
                                                q_sb[a + 32:a + 64, :])
                            nc.gpsimd.dma_start(qsw[a + 32:a + 64, :],
                                                q_sb[a:a + 32, :])
                        t1 = tmpp.tile([128, N], bf16, tag="t1")
                        nc.vector.tensor_mul(t1[:], q_sb[:], ca_bf[:])
                        t2 = tmpp.tile([128, N], bf16, tag="t2")
                        nc.vector.tensor_mul(t2[:], qsw[:], cb_bf[:])
                        nc.vector.tensor_add(dst[:, m, :], t1[:], t2[:])

                # ---- V projection (layout [token, 12*(64+1)]) ----
                v_sb = vp.tile([128, 5, H * 65], bf16, tag="v")
                for c, (ks, pr) in enumerate(KC):
                    vslab = v_sb[:, c, :].rearrange("p (h x) -> p h x", x=65)
                    pv = psum_pv(128, E)
                    for ns, nw in ET:
                        for kk in range(KO):
                            nc.tensor.matmul(
                                pv[:pr, ns:ns + nw],
                                lhsT=xbf[:, kk, t0 + ks:t0 + ks + pr],
                                rhs=wtiles["v"][:, kk, ns:ns + nw],
                                start=(kk == 0), stop=(kk == KO - 1),
                            )
                    nc.vector.tensor_copy(
                        vslab[:pr, :, 0:64],
                        pv[:pr, :].rearrange("p (h d) -> p h d", d=64),
                    )
                    nc.vector.memset(vslab[:pr, :, 64:65], 1.0)

                # ---- attention, head pairs interleaved (normalization
                # deferred).  The odd head's energy/PV matmuls read SBUF
                # partitions 64..127 -> tile_position row group 64, so the
                # pair's matmuls can run concurrently on the PE array.
                ot_sb = otp.tile([128, KO, N], bf16, tag="ot")
                for mc in range(KO):
                    pvos = [psum_pv(65, N), psum_pv(65, N)]
                    for c, (ks, pr) in enumerate(KC):
                        exps = []
                        for hh in range(2):
                            po = hh * 64
                            stp = psum(pr, N)
                            for ns, nw in NT:
                                nc.tensor.matmul(
                                    stp[:, ns:ns + nw],
                                    lhsT=kro[po:po + 64, mc, ks:ks + pr],
                                    rhs=qro[po:po + 64, mc, ns:ns + nw],
                                    start=True, stop=True,
                                )
                            exp_sb = expp.tile([128, N], bf16, tag="exp")
                            nc.scalar.activation(exp_sb[:pr, :], stp[:, :],
                                                 Exp, scale=SCALE)
                            exps.append(exp_sb)
                        for hh in range(2):
                            h = 2 * mc + hh
                            for ns, nw in NT:
                                nc.tensor.matmul(
                                    pvos[hh][:, ns:ns + nw],
                                    lhsT=v_sb[:pr, c, h * 65:h * 65 + 65],
                                    rhs=exps[hh][:pr, ns:ns + nw],
                                    start=(c == 0), stop=(c == 4),
                                )
                    for hh in range(2):
                        po = hh * 64
                        pvo = pvos[hh]
                        # copy PV result (ACT) + reciprocal of the denominator
                        # row (DVE, partition 64 -> 0) in parallel to free the
                        # PSUM slot; gpsimd broadcast + normalize trail off
                        # the critical path
                        nc.scalar.activation(ot_sb[po:po + 64, mc, :],
                                             pvo[:64, :], Copy)
                        rcp = smallp.tile([1, N], bf16, tag="rcp")
                        with nc.allow_low_precision(
                                reason="softmax denom reciprocal in bf16; "
                                       "same error class as bf16 operands"):
                            nc.vector.reciprocal(rcp[0:1, :], pvo[64:65, :])
                        rb_sb = tmpp.tile([128, N], bf16, tag="rb")
                        nc.gpsimd.partition_broadcast(rb_sb[:], rcp[0:1, :])
                        nc.vector.tensor_mul(ot_sb[po:po + 64, mc, :],
                                             ot_sb[po:po + 64, mc, :],
                                             rb_sb[po:po + 64, :])

                # ---- output projection [token, e] + bias ----
                for c, (ks, pr) in enumerate(KC):
                    osb = outsbp.tile([128, E], f32, tag="osb")
                    po_ = psum_pv(128, E)
                    for ns, nw in ET:
                        for kk in range(KO):
                            nc.tensor.matmul(
                                po_[:pr, ns:ns + nw],
                                lhsT=ot_sb[:, kk, ks:ks + pr],
                                rhs=wtiles["p"][:, kk, ns:ns + nw],
                                start=(kk == 0), stop=False,
                            )
                        nc.tensor.matmul(
                            po_[:pr, ns:ns + nw],
                            lhsT=ones_sb[0:1, 0:pr],
                            rhs=bpe_sb[:, ns:ns + nw],
                            start=False, stop=True,
                        )
                    nc.vector.tensor_copy(osb[:pr, :], po_[:pr, :])
                    nc.sync.dma_start(out_d[t0 + ks:t0 + ks + pr, :],
                                      osb[:pr, :])

    nc.compile()
    return nc


def _prepare_inputs(x, pe, Wq, bq, Wk, bk, Wv, bv, Wp, bp):
    perm = _head_perm()
    ca, cb = _rope_coeffs(np.asarray(pe, np.float32))
    wqT = np.ascontiguousarray(np.asarray(Wq, np.float32)[perm].T)
    wkT = np.ascontiguousarray(np.asarray(Wk, np.float32)[perm].T)
    wvT = np.ascontiguousarray(np.asarray(Wv, np.float32).T)
    wpT = np.ascontiguousarray(np.asarray(Wp, np.float32).T)
    bqp = np.ascontiguousarray(np.asarray(bq, np.float32)[perm].reshape(KO, 128).T)
    bkp = np.ascontiguousarray(np.asarray(bk, np.float32)[perm].reshape(KO, 128).T)
    bpe = (np.asarray(bp, np.float32)
           + np.asarray(Wp, np.float32) @ np.asarray(bv, np.float32))
    shared = {
        "wqT": wqT, "wkT": wkT, "wvT": wvT, "wpT": wpT,
        "bqp": bqp, "bkp": bkp, "bpe": bpe.reshape(1, E),
        "ca": ca, "cb": cb,
    }
    x = np.asarray(x, np.float32)
    in_maps = []
    for c in range(NCORES):
        xs = x[c * BL:(c + 1) * BL].reshape(T, E)
        m = dict(shared)
        m["xT"] = np.ascontiguousarray(xs.T)
        in_maps.append(m)
    return in_maps


def kernel(**inputs):
    from concourse.bass_utils import run_bass_kernel_spmd

    if "nc" not in _CACHE:
        _CACHE["nc"] = _build_bass()
    nc = _CACHE["nc"]
    in_maps = _prepare_inputs(**inputs)
    res = run_bass_kernel_spmd(nc, in_maps, core_ids=list(range(NCORES)))
    outs = [res.results[c]["out"].reshape(BL, N, E) for c in range(NCORES)]
    return np.concatenate(outs, axis=0)


# revision 36
# speedup vs baseline: 7.1810x; 7.1810x over previous
"""Multi-head attention (ViT-style, RoPE) Trainium2 Bass kernel.

Problem: x[32,577,768], 12 heads, d=64, RoPE on tokens 1..576, softmax,
output projection.  Data-parallel over batch across 8 NeuronCores
(4 batches per core).  All matmuls in bf16 with fp32 PSUM accumulation.

Layout strategy per core (b_loc=4, n=577, e=768):
  xT      [768, 2308]  (tokens of 4 batches concatenated, transposed)
  Q^T,K^T [e_out, tok] computed directly (lhsT = W^T tiles, rhs = xT tiles)
  RoPE in a permuted head layout (per head: rows 0..31 = even comps,
  32..63 = odd comps) so the rotation becomes
      q_rope = CA * q + CB * (P_swap @ q)
  with CA/CB [128,577] host-precomputed coefficient tiles (token 0 rows are
  identity) and P_swap a constant 128x128 permutation done on the PE.
  energy^T[k,q] = (K^T).T @ Q^T per (b,h) in 128-row k chunks; exp on ACT
  (scale=1/sqrt(768) folded in); PV via lhsT=[V | 1] augmented with a ones
  column so row 64 of the PV accumulator is the softmax denominator.
  Normalization: reciprocal on DVE, broadcast 1->64 rows via rank-1 matmul,
  multiply on DVE.  Output projection back to [tok, e] layout with bias
  added as a rank-1 (ones x bias) accumulation matmul.
  Biases: bq/bk via ACT bias during PSUM->SBUF copy; bv folded into the
  output bias on host (out += Wp @ bv, exact because softmax weights sum
  to 1); bp via the rank-1 matmul.
"""

import numpy as np

H = 12
E = 768
D = 64
N = 577
NCORES = 8
B = 32
BL = B // NCORES          # batches per core
T = BL * N                # tokens per core
KO = E // 128             # 6 contraction chunks
SCALE = 1.0 / np.sqrt(np.float32(E))

_CACHE = {}


def _head_perm():
    perm = np.empty(E, np.int64)
    j = np.arange(64)
    local = np.where(j < 32, 2 * j, 2 * (j - 32) + 1)
    for h in range(H):
        perm[h * 64 + j] = h * 64 + local
    return perm


def _rope_coeffs(pe):
    # CA/CB [128, 577] in the permuted layout; pattern identical for the two
    # heads in a 128-partition chunk.
    ca = np.zeros((128, N), np.float32)
    cb = np.zeros((128, N), np.float32)
    ca[:, 0] = 1.0
    for hh in range(2):
        base = hh * 64
        j = np.arange(32)
        # even-output rows (local j < 32): out = pe[...,0,0]*q + pe[...,0,1]*qswap
        ca[base + j, 1:] = pe[:, :, 0, 0].T
        cb[base + j, 1:] = pe[:, :, 0, 1].T
        # odd-output rows: q holds x_odd -> coeff pe[...,1,1]; qswap holds x_even
        ca[base + 32 + j, 1:] = pe[:, :, 1, 1].T
        cb[base + 32 + j, 1:] = pe[:, :, 1, 0].T
    return ca, cb


def _build_bass(reps=1):
    import concourse.bass as bass
    import concourse.mybir as mybir
    import concourse.tile as tile
    from concourse import bacc

    f32 = mybir.dt.float32
    bf16 = mybir.dt.bfloat16

    nc = bacc.Bacc("TRN2", target_bir_lowering=False, debug=False,
                   num_devices=NCORES)

    xT = nc.dram_tensor("xT", [E, T], f32, kind="ExternalInput")
    wq = nc.dram_tensor("wqT", [E, E], f32, kind="ExternalInput")
    wk = nc.dram_tensor("wkT", [E, E], f32, kind="ExternalInput")
    wv = nc.dram_tensor("wvT", [E, E], f32, kind="ExternalInput")
    wp = nc.dram_tensor("wpT", [E, E], f32, kind="ExternalInput")
    bqd = nc.dram_tensor("bqp", [128, KO], f32, kind="ExternalInput")
    bkd = nc.dram_tensor("bkp", [128, KO], f32, kind="ExternalInput")
    bped = nc.dram_tensor("bpe", [1, E], f32, kind="ExternalInput")
    cad = nc.dram_tensor("ca", [128, N], f32, kind="ExternalInput")
    cbd = nc.dram_tensor("cb", [128, N], f32, kind="ExternalInput")
    out_d = nc.dram_tensor("out", [T, E], f32, kind="ExternalOutput")

    NT = [(0, 512), (512, 65)]        # token free-dim tiles within a batch
    ET = [(0, 512), (512, 256)]       # e_out free-dim tiles
    KC = [(c * 128, 128 if c < 4 else N - 512) for c in range(5)]  # k chunks

    with tile.TileContext(nc) as tc:
        with (
            tc.tile_pool(name="const", bufs=1) as constp,
            tc.tile_pool(name="stage", bufs=2) as stagep,
            tc.tile_pool(name="persist", bufs=1) as persp,
            tc.tile_pool(name="qk", bufs=2) as qkp,
            tc.tile_pool(name="vpool", bufs=2) as vp,
            tc.tile_pool(name="otp", bufs=2) as otp,
            tc.tile_pool(name="tmp", bufs=3) as tmpp,
            tc.tile_pool(name="expp", bufs=3) as expp,
            tc.tile_pool(name="small", bufs=2) as smallp,
            tc.tile_pool(name="outsb", bufs=2) as outsbp,
            tc.tile_pool(name="ps", bufs=2, space="PSUM") as psp,
            tc.tile_pool(name="pvps", bufs=2, space="PSUM") as pvpsp,
        ):
            def psum(p, f):
                # 2-bank slot, shared by ST/projection/output tiles
                t = psp.tile([128, 768], f32, tag="ps")
                return t[:p, :f]

            def psum_pv(p, f):
                # dedicated rotation for PV accumulators
                t = pvpsp.tile([128, 768], f32, tag="pv")
                return t[:p, :f]

            # ---- load + cast constants ----
            # x and Wq chunks first (the first projection needs them), the
            # rest behind.
            xbf = persp.tile([128, KO, T], bf16)
            wtiles = {n: persp.tile([128, KO, E], bf16, tag=f"w{n}",
                                    name=f"w{n}")
                      for n in ("q", "k", "v", "p")}
            wdram = {"q": wq, "k": wk, "v": wv, "p": wp}
            for ko in range(KO):
                st = stagep.tile([128, T], f32, tag="stx")
                nc.sync.dma_start(st[:], xT[ko * 128:(ko + 1) * 128, :])
                nc.vector.tensor_copy(xbf[:, ko, :], st[:])
                sw = stagep.tile([128, E], f32, tag="stw")
                nc.sync.dma_start(sw[:], wq[ko * 128:(ko + 1) * 128, :])
                nc.vector.tensor_copy(wtiles["q"][:, ko, :], sw[:])
            for name in ("k", "v", "p"):
                wt, wd = wtiles[name], wdram[name]
                for ko in range(KO):
                    st = stagep.tile([128, E], f32, tag="stw")
                    nc.sync.dma_start(st[:], wd[ko * 128:(ko + 1) * 128, :])
                    nc.vector.tensor_copy(wt[:, ko, :], st[:])

            ca_bf = constp.tile([128, N], bf16)
            cb_bf = constp.tile([128, N], bf16)
            for dst, src in ((ca_bf, cad), (cb_bf, cbd)):
                st = stagep.tile([128, N], f32, tag="stc")
                nc.sync.dma_start(st[:], src[:, :])
                nc.vector.tensor_copy(dst[:], st[:])
            bq_sb = constp.tile([128, KO], f32, tag="bq")
            bk_sb = constp.tile([128, KO], f32, tag="bk")
            nc.sync.dma_start(bq_sb[:], bqd[:, :])
            nc.sync.dma_start(bk_sb[:], bkd[:, :])
            bpe_sb = constp.tile([1, E], bf16, tag="bpe")
            st = stagep.tile([1, E], f32, tag="stb")
            nc.sync.dma_start(st[:], bped[:, :])
            nc.vector.tensor_copy(bpe_sb[:], st[:])
            ones_sb = constp.tile([65, 128], bf16, tag="ones")
            nc.vector.memset(ones_sb[:], 1.0)

            Ident = mybir.ActivationFunctionType.Identity
            Copy = mybir.ActivationFunctionType.Copy
            Exp = mybir.ActivationFunctionType.Exp

            for b in [bb % BL for bb in range(reps * BL)]:
                t0 = b * N

                # ---- Q^T / K^T projection + RoPE ----
                qro = qkp.tile([128, KO, N], bf16, tag="qro")
                kro = qkp.tile([128, KO, N], bf16, tag="kro")
                for wname, bias_sb, dst in (("q", bq_sb, qro), ("k", bk_sb, kro)):
                    wt = wtiles[wname]
                    for m in range(KO):
                        q_sb = tmpp.tile([128, N], bf16, tag="q_sb")
                        pq = psum_pv(128, N)
                        for ns, nw in NT:
                            for kk in range(KO):
                                nc.tensor.matmul(
                                    pq[:, ns:ns + nw],
                                    lhsT=wt[:, kk, m * 128:(m + 1) * 128],
                                    rhs=xbf[:, kk, t0 + ns:t0 + ns + nw],
                                    start=(kk == 0), stop=(kk == KO - 1),
                                )
                        nc.scalar.activation(q_sb[:], pq[:, :], Ident,
                                             bias=bias_sb[:, m:m + 1])
                        # swap even/odd 32-row blocks via SBUF->SBUF DMA
                        qsw = tmpp.tile([128, N], bf16, tag="qsw")
                        for a in (0, 64):
                            nc.gpsimd.dma_start(qsw[a:a + 32, :],
                                                q_sb[a + 32:a + 64, :])
                            nc.gpsimd.dma_start(qsw[a + 32:a + 64, :],
                                                q_sb[a:a + 32, :])
                        t1 = tmpp.tile([128, N], bf16, tag="t1")
                        nc.vector.tensor_mul(t1[:], q_sb[:], ca_bf[:])
                        t2 = tmpp.tile([128, N], bf16, tag="t2")
                        nc.vector.tensor_mul(t2[:], qsw[:], cb_bf[:])
                        nc.vector.tensor_add(dst[:, m, :], t1[:], t2[:])

                # ---- V projection (layout [token, 12*(64+1)]) ----
                v_sb = vp.tile([128, 5, H * 65], bf16, tag="v")
                for c, (ks, pr) in enumerate(KC):
                    vslab = v_sb[:, c, :].rearrange("p (h x) -> p h x", x=65)
                    pv = psum_pv(128, E)
                    for ns, nw in ET:
                        for kk in range(KO):
                            nc.tensor.matmul(
                                pv[:pr, ns:ns + nw],
                                lhsT=xbf[:, kk, t0 + ks:t0 + ks + pr],
                                rhs=wtiles["v"][:, kk, ns:ns + nw],
                                start=(kk == 0), stop=(kk == KO - 1),
                            )
                    nc.vector.tensor_copy(
                        vslab[:pr, :, 0:64],
                        pv[:pr, :].rearrange("p (h d) -> p h d", d=64),
                    )
                    nc.vector.memset(vslab[:pr, :, 64:65], 1.0)

                # ---- attention, head pairs interleaved (normalization
                # deferred).  The odd head's energy/PV matmuls read SBUF
                # partitions 64..127 -> tile_position row group 64, so the
                # pair's matmuls can run concurrently on the PE array.
                ot_sb = otp.tile([128, KO, N], bf16, tag="ot")
                for mc in range(KO):
                    pvos = [psum_pv(65, N), psum_pv(65, N)]
                    for c, (ks, pr) in enumerate(KC):
                        exps = []
                        for hh in range(2):
                            po = hh * 64
                            stp = psum(pr, N)
                            for ns, nw in NT:
                                nc.tensor.matmul(
                                    stp[:, ns:ns + nw],
                                    lhsT=kro[po:po + 64, mc, ks:ks + pr],
                                    rhs=qro[po:po + 64, mc, ns:ns + nw],
                                    start=True, stop=True,
                                )
                            exp_sb = expp.tile([128, N], bf16, tag="exp")
                            nc.scalar.activation(exp_sb[:pr, :], stp[:, :],
                                                 Exp, scale=SCALE)
                            exps.append(exp_sb)
                        for hh in range(2):
                            h = 2 * mc + hh
                            for ns, nw in NT:
                                nc.tensor.matmul(
                                    pvos[hh][:, ns:ns + nw],
                                    lhsT=v_sb[:pr, c, h * 65:h * 65 + 65],
                                    rhs=exps[hh][:pr, ns:ns + nw],
                                    start=(c == 0), stop=(c == 4),
                                )
                    for hh in range(2):
                        po = hh * 64
                        pvo = pvos[hh]
                        # copy PV result (ACT) + reciprocal of the denominator
                        # row (DVE, partition 64 -> 0) in parallel to free the
                        # PSUM slot; gpsimd broadcast + normalize trail off
                        # the critical path
                        nc.scalar.activation(ot_sb[po:po + 64, mc, :],
                                             pvo[:64, :], Copy)
                        rcp = smallp.tile([1, N], bf16, tag="rcp")
                        with nc.allow_low_precision(
                                reason="softmax denom reciprocal in bf16; "
                                       "same error class as bf16 operands"):
                            nc.vector.reciprocal(rcp[0:1, :], pvo[64:65, :])
                        rb_sb = tmpp.tile([128, N], bf16, tag="rb")
                        nc.gpsimd.partition_broadcast(rb_sb[:], rcp[0:1, :])
                        nc.vector.tensor_mul(ot_sb[po:po + 64, mc, :],
                                             ot_sb[po:po + 64, mc, :],
                                             rb_sb[po:po + 64, :])

                # ---- output projection [token, e] + bias ----
                for c, (ks, pr) in enumerate(KC):
                    osb = outsbp.tile([128, E], f32, tag="osb")
                    po_ = psum_pv(128, E)
                    for ns, nw in ET:
                        for kk in range(KO):
                            nc.tensor.matmul(
                                po_[:pr, ns:ns + nw],
                                lhsT=ot_sb[:, kk, ks:ks + pr],
                                rhs=wtiles["p"][:, kk, ns:ns + nw],
                                start=(kk == 0), stop=False,
                            )
                        nc.tensor.matmul(
                            po_[:pr, ns:ns + nw],
                            lhsT=ones_sb[0:1, 0:pr],
                            rhs=bpe_sb[:, ns:ns + nw],
                            start=False, stop=True,
                        )
                    nc.vector.tensor_copy(osb[:pr, :], po_[:pr, :])
                    nc.sync.dma_start(out_d[t0 + ks:t0 + ks + pr, :],
                                      osb[:pr, :])

    nc.compile()
    return nc


def _prepare_inputs(x, pe, Wq, bq, Wk, bk, Wv, bv, Wp, bp):
    perm = _head_perm()
    ca, cb = _rope_coeffs(np.asarray(pe, np.float32))
    wqT = np.ascontiguousarray(np.asarray(Wq, np.float32)[perm].T)
    wkT = np.ascontiguousarray(np.asarray(Wk, np.float32)[perm].T)
    wvT = np.ascontiguousarray(np.asarray(Wv, np.float32).T)
    wpT = np.ascontiguousarray(np.asarray(Wp, np.float32).T)
    bqp = np.ascontiguousarray(np.asarray(bq, np.float32)[perm].reshape(KO, 128).T)
    bkp = np.ascontiguousarray(np.asarray(bk, np.float32)[perm].reshape(KO, 128).T)
    bpe = (np.asarray(bp, np.float32)
           + np.asarray(Wp, np.float32) @ np.asarray(bv, np.float32))
    shared = {
        "wqT": wqT, "wkT": wkT, "wvT": wvT, "wpT": wpT,
        "bqp": bqp, "bkp": bkp, "bpe": bpe.reshape(1, E),
        "ca": ca, "cb": cb,
    }
    x = np.asarray(x, np.float32)
    in_maps = []
    for c in range(NCORES):
        xs = x[c * BL:(c + 1) * BL].reshape(T, E)
        m = dict(shared)
        m["xT"] = np.ascontiguousarray(xs.T)
        in_maps.append(m)
    return in_maps


def kernel(**inputs):
    from concourse.bass_utils import run_bass_kernel_spmd

    if "nc" not in _CACHE:
        _CACHE["nc"] = _build_bass()
    nc = _CACHE["nc"]
    in_maps = _prepare_inputs(**inputs)
    res = run_bass_kernel_spmd(nc, in_maps, core_ids=list(range(NCORES)))
    outs = [res.results[c]["out"].reshape(BL, N, E) for c in range(NCORES)]
    return np.concatenate(outs, axis=0)


# revision 37
# speedup vs baseline: 7.2541x; 1.0102x over previous
"""Multi-head attention (ViT-style, RoPE) Trainium2 Bass kernel.

Problem: x[32,577,768], 12 heads, d=64, RoPE on tokens 1..576, softmax,
output projection.  Data-parallel over batch across 8 NeuronCores
(4 batches per core).  All matmuls in bf16 with fp32 PSUM accumulation.

Layout strategy per core (b_loc=4, n=577, e=768):
  xT      [768, 2308]  (tokens of 4 batches concatenated, transposed)
  Q^T,K^T [e_out, tok] computed directly (lhsT = W^T tiles, rhs = xT tiles)
  RoPE in a permuted head layout (per head: rows 0..31 = even comps,
  32..63 = odd comps) so the rotation becomes
      q_rope = CA * q + CB * (P_swap @ q)
  with CA/CB [128,577] host-precomputed coefficient tiles (token 0 rows are
  identity) and P_swap a constant 128x128 permutation done on the PE.
  energy^T[k,q] = (K^T).T @ Q^T per (b,h) in 128-row k chunks; exp on ACT
  (scale=1/sqrt(768) folded in); PV via lhsT=[V | 1] augmented with a ones
  column so row 64 of the PV accumulator is the softmax denominator.
  Normalization: reciprocal on DVE, broadcast 1->64 rows via rank-1 matmul,
  multiply on DVE.  Output projection back to [tok, e] layout with bias
  added as a rank-1 (ones x bias) accumulation matmul.
  Biases: bq/bk via ACT bias during PSUM->SBUF copy; bv folded into the
  output bias on host (out += Wp @ bv, exact because softmax weights sum
  to 1); bp via the rank-1 matmul.
"""

import numpy as np

H = 12
E = 768
D = 64
N = 577
NCORES = 8
B = 32
BL = B // NCORES          # batches per core
T = BL * N                # tokens per core
KO = E // 128             # 6 contraction chunks
SCALE = 1.0 / np.sqrt(np.float32(E))

_CACHE = {}


def _head_perm():
    perm = np.empty(E, np.int64)
    j = np.arange(64)
    local = np.where(j < 32, 2 * j, 2 * (j - 32) + 1)
    for h in range(H):
        perm[h * 64 + j] = h * 64 + local
    return perm


def _rope_coeffs(pe):
    # CA/CB [128, 577] in the permuted layout; pattern identical for the two
    # heads in a 128-partition chunk.
    ca = np.zeros((128, N), np.float32)
    cb = np.zeros((128, N), np.float32)
    ca[:, 0] = 1.0
    for hh in range(2):
        base = hh * 64
        j = np.arange(32)
        # even-output rows (local j < 32): out = pe[...,0,0]*q + pe[...,0,1]*qswap
        ca[base + j, 1:] = pe[:, :, 0, 0].T
        cb[base + j, 1:] = pe[:, :, 0, 1].T
        # odd-output rows: q holds x_odd -> coeff pe[...,1,1]; qswap holds x_even
        ca[base + 32 + j, 1:] = pe[:, :, 1, 1].T
        cb[base + 32 + j, 1:] = pe[:, :, 1, 0].T
    return ca, cb


def _build_bass(reps=1):
    import concourse.bass as bass
    import concourse.mybir as mybir
    import concourse.tile as tile
    from concourse import bacc

    f32 = mybir.dt.float32
    bf16 = mybir.dt.bfloat16

    nc = bacc.Bacc("TRN2", target_bir_lowering=False, debug=False,
                   num_devices=NCORES)

    xT = nc.dram_tensor("xT", [E, T], f32, kind="ExternalInput")
    wq = nc.dram_tensor("wqT", [E, E], f32, kind="ExternalInput")
    wk = nc.dram_tensor("wkT", [E, E], f32, kind="ExternalInput")
    wv = nc.dram_tensor("wvT", [E, E], f32, kind="ExternalInput")
    wp = nc.dram_tensor("wpT", [E, E], f32, kind="ExternalInput")
    bqd = nc.dram_tensor("bqp", [128, KO], f32, kind="ExternalInput")
    bkd = nc.dram_tensor("bkp", [128, KO], f32, kind="ExternalInput")
    bped = nc.dram_tensor("bpe", [1, E], f32, kind="ExternalInput")
    cad = nc.dram_tensor("ca", [128, N], f32, kind="ExternalInput")
    cbd = nc.dram_tensor("cb", [128, N], f32, kind="ExternalInput")
    out_d = nc.dram_tensor("out", [T, E], f32, kind="ExternalOutput")

    NT = [(0, 512), (512, 65)]        # token free-dim tiles within a batch
    ET = [(0, 512), (512, 256)]       # e_out free-dim tiles
    KC = [(c * 128, 128 if c < 4 else N - 512) for c in range(5)]  # k chunks

    with tile.TileContext(nc) as tc:
        with (
            tc.tile_pool(name="const", bufs=1) as constp,
            tc.tile_pool(name="stage", bufs=2) as stagep,
            tc.tile_pool(name="persist", bufs=1) as persp,
            tc.tile_pool(name="qk", bufs=2) as qkp,
            tc.tile_pool(name="vpool", bufs=2) as vp,
            tc.tile_pool(name="otp", bufs=2) as otp,
            tc.tile_pool(name="tmp", bufs=3) as tmpp,
            tc.tile_pool(name="expp", bufs=3) as expp,
            tc.tile_pool(name="small", bufs=3) as smallp,
            tc.tile_pool(name="outsb", bufs=3) as outsbp,
            tc.tile_pool(name="ps", bufs=2, space="PSUM") as psp,
            tc.tile_pool(name="pvps", bufs=2, space="PSUM") as pvpsp,
        ):
            def psum(p, f):
                # 2-bank slot, shared by ST/projection/output tiles
                t = psp.tile([128, 768], f32, tag="ps")
                return t[:p, :f]

            def psum_pv(p, f):
                # dedicated rotation for PV accumulators
                t = pvpsp.tile([128, 768], f32, tag="pv")
                return t[:p, :f]

            # ---- load + cast constants ----
            # x and Wq chunks first (the first projection needs them), the
            # rest behind.
            xbf = persp.tile([128, KO, T], bf16)
            wtiles = {n: persp.tile([128, KO, E], bf16, tag=f"w{n}",
                                    name=f"w{n}")
                      for n in ("q", "k", "v", "p")}
            wdram = {"q": wq, "k": wk, "v": wv, "p": wp}
            for ko in range(KO):
                st = stagep.tile([128, T], f32, tag="stx")
                nc.sync.dma_start(st[:], xT[ko * 128:(ko + 1) * 128, :])
                nc.vector.tensor_copy(xbf[:, ko, :], st[:])
                sw = stagep.tile([128, E], f32, tag="stw")
                nc.sync.dma_start(sw[:], wq[ko * 128:(ko + 1) * 128, :])
                nc.vector.tensor_copy(wtiles["q"][:, ko, :], sw[:])
            for name in ("k", "v", "p"):
                wt, wd = wtiles[name], wdram[name]
                for ko in range(KO):
                    st = stagep.tile([128, E], f32, tag="stw")
                    nc.sync.dma_start(st[:], wd[ko * 128:(ko + 1) * 128, :])
                    nc.vector.tensor_copy(wt[:, ko, :], st[:])

            ca_bf = constp.tile([128, N], bf16)
            cb_bf = constp.tile([128, N], bf16)
            for dst, src in ((ca_bf, cad), (cb_bf, cbd)):
                st = stagep.tile([128, N], f32, tag="stc")
                nc.sync.dma_start(st[:], src[:, :])
                nc.vector.tensor_copy(dst[:], st[:])
            bq_sb = constp.tile([128, KO], f32, tag="bq")
            bk_sb = constp.tile([128, KO], f32, tag="bk")
            nc.sync.dma_start(bq_sb[:], bqd[:, :])
            nc.sync.dma_start(bk_sb[:], bkd[:, :])
            bpe_sb = constp.tile([1, E], bf16, tag="bpe")
            st = stagep.tile([1, E], f32, tag="stb")
            nc.sync.dma_start(st[:], bped[:, :])
            nc.vector.tensor_copy(bpe_sb[:], st[:])
            ones_sb = constp.tile([65, 128], bf16, tag="ones")
            nc.vector.memset(ones_sb[:], 1.0)

            Ident = mybir.ActivationFunctionType.Identity
            Copy = mybir.ActivationFunctionType.Copy
            Exp = mybir.ActivationFunctionType.Exp

            for b in [bb % BL for bb in range(reps * BL)]:
                t0 = b * N

                # ---- Q^T / K^T projection + RoPE ----
                qro = qkp.tile([128, KO, N], bf16, tag="qro")
                kro = qkp.tile([128, KO, N], bf16, tag="kro")
                for wname, bias_sb, dst in (("q", bq_sb, qro), ("k", bk_sb, kro)):
                    wt = wtiles[wname]
                    for m in range(KO):
                        q_sb = tmpp.tile([128, N], bf16, tag="q_sb")
                        pq = psum_pv(128, N)
                        for ns, nw in NT:
                            for kk in range(KO):
                                nc.tensor.matmul(
                                    pq[:, ns:ns + nw],
                                    lhsT=wt[:, kk, m * 128:(m + 1) * 128],
                                    rhs=xbf[:, kk, t0 + ns:t0 + ns + nw],
                                    start=(kk == 0), stop=(kk == KO - 1),
                                )
                        nc.scalar.activation(q_sb[:], pq[:, :], Ident,
                                             bias=bias_sb[:, m:m + 1])
                        # swap even/odd 32-row blocks via SBUF->SBUF DMA
                        qsw = tmpp.tile([128, N], bf16, tag="qsw")
                        for a in (0, 64):
                            nc.gpsimd.dma_start(qsw[a:a + 32, :],
                                                q_sb[a + 32:a + 64, :])
                            nc.gpsimd.dma_start(qsw[a + 32:a + 64, :],
                                                q_sb[a:a + 32, :])
                        t1 = tmpp.tile([128, N], bf16, tag="t1")
                        nc.vector.tensor_mul(t1[:], q_sb[:], ca_bf[:])
                        t2 = tmpp.tile([128, N], bf16, tag="t2")
                        nc.vector.tensor_mul(t2[:], qsw[:], cb_bf[:])
                        nc.vector.tensor_add(dst[:, m, :], t1[:], t2[:])

                # ---- V projection (layout [token, 12*(64+1)]) ----
                v_sb = vp.tile([128, 5, H * 65], bf16, tag="v")
                for c, (ks, pr) in enumerate(KC):
                    vslab = v_sb[:, c, :].rearrange("p (h x) -> p h x", x=65)
                    pv = psum_pv(128, E)
                    for ns, nw in ET:
                        for kk in range(KO):
                            nc.tensor.matmul(
                                pv[:pr, ns:ns + nw],
                                lhsT=xbf[:, kk, t0 + ks:t0 + ks + pr],
                                rhs=wtiles["v"][:, kk, ns:ns + nw],
                                start=(kk == 0), stop=(kk == KO - 1),
                            )
                    nc.vector.tensor_copy(
                        vslab[:pr, :, 0:64],
                        pv[:pr, :].rearrange("p (h d) -> p h d", d=64),
                    )
                    nc.vector.memset(vslab[:pr, :, 64:65], 1.0)

                # ---- attention, head pairs interleaved (normalization
                # deferred).  The odd head's energy/PV matmuls read SBUF
                # partitions 64..127 -> tile_position row group 64, so the
                # pair's matmuls can run concurrently on the PE array.
                ot_sb = otp.tile([128, KO, N], bf16, tag="ot")
                for mc in range(KO):
                    pvos = [psum_pv(65, N), psum_pv(65, N)]
                    for c, (ks, pr) in enumerate(KC):
                        exps = []
                        for hh in range(2):
                            po = hh * 64
                            stp = psum(pr, N)
                            for ns, nw in NT:
                                nc.tensor.matmul(
                                    stp[:, ns:ns + nw],
                                    lhsT=kro[po:po + 64, mc, ks:ks + pr],
                                    rhs=qro[po:po + 64, mc, ns:ns + nw],
                                    start=True, stop=True,
                                )
                            exp_sb = expp.tile([128, N], bf16, tag="exp")
                            nc.scalar.activation(exp_sb[:pr, :], stp[:, :],
                                                 Exp, scale=SCALE)
                            exps.append(exp_sb)
                        for hh in range(2):
                            h = 2 * mc + hh
                            for ns, nw in NT:
                                nc.tensor.matmul(
                                    pvos[hh][:, ns:ns + nw],
                                    lhsT=v_sb[:pr, c, h * 65:h * 65 + 65],
                                    rhs=exps[hh][:pr, ns:ns + nw],
                                    start=(c == 0), stop=(c == 4),
                                )
                    for hh in range(2):
                        po = hh * 64
                        pvo = pvos[hh]
                        # copy PV result (ACT) + reciprocal of the denominator
                        # row (DVE, partition 64 -> 0) in parallel to free the
                        # PSUM slot; gpsimd broadcast + normalize trail off
                        # the critical path
                        nc.scalar.activation(ot_sb[po:po + 64, mc, :],
                                             pvo[:64, :], Copy)
                        rcp = smallp.tile([1, N], bf16, tag="rcp")
                        with nc.allow_low_precision(
                                reason="softmax denom reciprocal in bf16; "
                                       "same error class as bf16 operands"):
                            nc.vector.reciprocal(rcp[0:1, :], pvo[64:65, :])
                        rb_sb = tmpp.tile([128, N], bf16, tag="rb")
                        nc.gpsimd.partition_broadcast(rb_sb[:], rcp[0:1, :])
                        nc.vector.tensor_mul(ot_sb[po:po + 64, mc, :],
                                             ot_sb[po:po + 64, mc, :],
                                             rb_sb[po:po + 64, :])

                # ---- output projection [token, e] + bias ----
                for c, (ks, pr) in enumerate(KC):
                    osb = outsbp.tile([128, E], f32, tag="osb")
                    po_ = psum_pv(128, E)
                    for ns, nw in ET:
                        for kk in range(KO):
                            nc.tensor.matmul(
                                po_[:pr, ns:ns + nw],
                                lhsT=ot_sb[:, kk, ks:ks + pr],
                                rhs=wtiles["p"][:, kk, ns:ns + nw],
                                start=(kk == 0), stop=False,
                            )
                        nc.tensor.matmul(
                            po_[:pr, ns:ns + nw],
                            lhsT=ones_sb[0:1, 0:pr],
                            rhs=bpe_sb[:, ns:ns + nw],
                            start=False, stop=True,
                        )
                    nc.vector.tensor_copy(osb[:pr, :], po_[:pr, :])
                    nc.sync.dma_start(out_d[t0 + ks:t0 + ks + pr, :],
                                      osb[:pr, :])

    nc.compile()
    return nc


def _prepare_inputs(x, pe, Wq, bq, Wk, bk, Wv, bv, Wp, bp):
    perm = _head_perm()
    ca, cb = _rope_coeffs(np.asarray(pe, np.float32))
    wqT = np.ascontiguousarray(np.asarray(Wq, np.float32)[perm].T)
    wkT = np.ascontiguousarray(np.asarray(Wk, np.float32)[perm].T)
    wvT = np.ascontiguousarray(np.asarray(Wv, np.float32).T)
    wpT = np.ascontiguousarray(np.asarray(Wp, np.float32).T)
    bqp = np.ascontiguousarray(np.asarray(bq, np.float32)[perm].reshape(KO, 128).T)
    bkp = np.ascontiguousarray(np.asarray(bk, np.float32)[perm].reshape(KO, 128).T)
    bpe = (np.asarray(bp, np.float32)
           + np.asarray(Wp, np.float32) @ np.asarray(bv, np.float32))
    shared = {
        "wqT": wqT, "wkT": wkT, "wvT": wvT, "wpT": wpT,
        "bqp": bqp, "bkp": bkp, "bpe": bpe.reshape(1, E),
        "ca": ca, "cb": cb,
    }
    x = np.asarray(x, np.float32)
    in_maps = []
    for c in range(NCORES):
        xs = x[c * BL:(c + 1) * BL].reshape(T, E)
        m = dict(shared)
        m["xT"] = np.ascontiguousarray(xs.T)
        in_maps.append(m)
    return in_maps


def kernel(**inputs):
    from concourse.bass_utils import run_bass_kernel_spmd

    if "nc" not in _CACHE:
        _CACHE["nc"] = _build_bass()
    nc = _CACHE["nc"]
    in_maps = _prepare_inputs(**inputs)
    res = run_bass_kernel_spmd(nc, in_maps, core_ids=list(range(NCORES)))
    outs = [res.results[c]["out"].reshape(BL, N, E) for c in range(NCORES)]
    return np.concatenate(outs, axis=0)
